# revision 61
# baseline (speedup 1.0000x reference)
"""Multi-head causal self-attention on 8 Trainium2 NeuronCores.

Problem: B=4, S=2048, D=1024, H=16 heads (dk=64), fp32 in/out, causal softmax.

Sharding: hybrid batch x head-group. Core c handles batch b = c//2 and head
group g = c%2 (8 heads = 512 dims). Each core computes QKV projections for
its head group, causal attention, and a partial output projection over its
512 context dims. The host sums the two bf16 partials per batch in fp32.

Device-side design (per core):
  - QKV projections in fp8e4m3 DoubleRow mode with hi+lo compensation:
    x and W split host-side into hi = e4m3(t), lo = e4m3(t - hi) (pow2
    pre-scales keep values in e4m3's normal range).  Per 128-slab, slots
    (x_hi,W_hi)+(x_lo,W_hi) give x~@W_hi in one 0.5-cycle/row matmul and
    slab pairs share (x_hi,W_lo) correction matmuls: 0.75x bf16 PE cost
    at ~bf16 accuracy.
  - Scores in fp16 via the zero-padded head-pair trick (q duplicated with
    the other head's partitions zeroed; full-128 contraction from base
    partition 0 — fp8 scores cost too much accuracy at dk=64).
  - exp split: heads 0-5 exact on ACT with fp8e4m3 output (scale=2^-14
    folds the projection scale chain, bias=-5ln2 keeps pt=exp(s)/32 under
    e4m3's max), heads 6-7 via a Schraudolph bit-trick on DVE
    (int16(A*s'+B) bitcast to f16; ~2% multiplicative error that largely
    cancels in the per-head softmax ratio).
  - P@V: heads 0-5 run fp8 DoubleRow with V split hi+lo at the drain
    (slots (pt,v_hi)+(pt,v_lo), 0.5 cycles/row); heads 6-7 fp16.  Ones
    columns make each head's denominator; per-head reciprocal normalize
    cancels all per-head pt scales exactly.
  - PSUM ctx accumulator zeroed by the start=True 2KB bank zeroing of the
    first PV matmul per bank (h=0/h=4 at kc=0) — no DVE memset.
  - ctx^T produced by DMA-engine transposes on the SP queue (no PE cost).
  - Schedule: forward q-tile sweep, projections and deferred output
    projections drained greedily into the attention loop as PE filler.
"""

import numpy as np
from contextlib import ExitStack

import concourse.bass as bass
import concourse.tile as tile
from concourse import bacc, mybir
from concourse.bass_utils import run_bass_kernel_spmd

B, S, D = 4, 2048, 1024
H16 = 16
DK = 64
G = 2               # head groups (cores per batch)
HD = D // G         # per-core head dims = 512 (8 heads)
NH = HD // DK       # heads per core = 8
NJ = NH // 2        # head pairs per core = 4
P = 128
NQS = S // P        # 16 q subtiles
NKC = S // P        # 16 k chunks
KO = D // P         # 8 contraction chunks for projections
QC = 512            # projection s-chunk
HC = 256            # DoubleRow rhs half-chunk (keeps moving free <= 512)
NHA = 6             # heads 0..5: exact exp on ACT -> e4m3 pt, DR P@V
NHS = NH - NHA      # heads 6..7: Schraudolph exp on DVE -> f16 pt, f16 P@V

F32 = mybir.dt.float32
F16 = mybir.dt.float16
BF16 = mybir.dt.bfloat16
E4 = mybir.dt.float8e4
I16 = mybir.dt.int16
DR = mybir.MatmulPerfMode.DoubleRow
EXP = mybir.ActivationFunctionType.Exp

XS = 8.0            # host pre-scale on x before e4m3 split
WS = 256.0          # host pre-scale on W_q/W_k/W_v before e4m3 split
# q/k drains stay at the raw psum scale 2048; scores psum s' = 2^22*8*s_ref
ACT_SCALE = 2.0 ** -25
# -6 ln2: pt = exp(s_ref)/64 — max score 9.07 -> 135 < e4m3 max 240, and
# the 2^-9 subnormal flush only drops weights below exp(-2.08) relative
# mass ~0.07%
ACT_BIAS = -4.1588830833596715
SCH_A = 1024.0 * 1.4426950408889634 * ACT_SCALE   # f16 Schraudolph mult
# fp16 exponent base (15-1)*1024 (pt = exp(s)/2 for these heads, uniform
# across ALL their tiles so the per-head softmax normalize cancels it),
# -44 minimax; i16 stays in [1748, 26315] over the whole score range
SCH_B = 14336.0 - 44.0
VS8 = 2.0 ** -8     # V drain rescale for the e4m3 hi/lo copy (scale 8)
ONES8 = 8.0         # ones value for e4m3 P@V heads (v8 = 8*v)
ONES16 = 2048.0     # ones value for f16 P@V heads (v16 = 2048*v)

_CACHE: dict = {}


def _emit(ctx: ExitStack, tc, x8a, x8b, wqh, wql, wkh, wkl, wvh, wvl, wo,
          tri, out):
    nc = tc.nc

    persist = ctx.enter_context(tc.tile_pool(name="persist", bufs=1))
    x8_sb = persist.tile([P, KO, 2, S], E4)
    wqh_sb = persist.tile([P, KO, HD], E4)
    wql_sb = persist.tile([P, KO, HD], E4)
    wkh_sb = persist.tile([P, KO, HD], E4)
    wkl_sb = persist.tile([P, KO, HD], E4)
    wvh_sb = persist.tile([P, KO, 2, HD], E4)   # hi duplicated host-side
    wvl_sb = persist.tile([P, KO, HD], E4)
    wo_sb = persist.tile([P, NJ, D], F16)
    # q stored twice with the other head's partitions zeroed so score
    # matmuls contract the full 128 partitions from base partition 0
    # (operands at base partition 64 fail on hardware)
    qt_ev = persist.tile([P, NJ, S], F16)
    qt_od = persist.tile([P, NJ, S], F16)
    kt_sb = persist.tile([P, NJ, S], F16)
    v8_sb = persist.tile([P, NKC, 2, NHA, DK + 1], E4)   # dim2 = (hi, lo)
    v16_sb = persist.tile([P, NKC, NH, DK + 1], F16)     # 8*v, all heads
    ctxT_sb = persist.tile([P, NJ, NQS, P], F16)
    tri_sb = persist.tile([P, 1, P], F16)
    bias_sb = persist.tile([P, 1], F32)
    warm_sb = persist.tile([P, 2], E4)

    x8a_r = x8a.rearrange("(o p) s -> p o s", p=P)
    x8b_r = x8b.rearrange("(o p) s -> p o s", p=P)
    wqh_r = wqh.rearrange("(o p) m -> p o m", p=P)
    wql_r = wql.rearrange("(o p) m -> p o m", p=P)
    wkh_r = wkh.rearrange("(o p) m -> p o m", p=P)
    wkl_r = wkl.rearrange("(o p) m -> p o m", p=P)
    wvh_r = wvh.rearrange("(o p) t m -> p o t m", p=P)
    wvl_r = wvl.rearrange("(o p) m -> p o m", p=P)
    wo_r = wo.rearrange("(j p) o -> p j o", p=P)

    with (
        tc.tile_pool(name="spps", bufs=2, space="PSUM") as spps,
        tc.tile_pool(name="cxps", bufs=1, space="PSUM") as cxps,
        tc.tile_pool(name="mixps", bufs=2, space="PSUM") as mixps,
        tc.tile_pool(name="ptp", bufs=3) as ptp,
        tc.tile_pool(name="cnp", bufs=2) as cnp,
        tc.tile_pool(name="rrp", bufs=2) as rrp,
        tc.tile_pool(name="osb", bufs=3) as osb,
    ):
        # ---- input DMAs.  The first K projection group (pair j=0, s-chunk
        # 0) consumes wk columns 0:128 (hi+lo) and x8 sc0 slab by slab, so
        # those land first: wk j0-hi on SP, j0-lo on the DVE queue, x8 sc0
        # hi on Pool / lo on ACT with the first two slabs in a small lead
        # copy (ACT is exp-free until the first q-tile).
        nc.sync.dma_start(wkh_sb[:, 0:2, 0:P], wkh_r[:, 0:2, 0:P])
        nc.sync.dma_start(wkl_sb[:, 0:2, 0:P], wkl_r[:, 0:2, 0:P])
        nc.sync.dma_start(wkh_sb[:, 2:KO, 0:P], wkh_r[:, 2:KO, 0:P])
        nc.sync.dma_start(wkl_sb[:, 2:KO, 0:P], wkl_r[:, 2:KO, 0:P])
        nc.gpsimd.dma_start(x8_sb[:, 0:2, 0, 0:QC], x8a_r[:, 0:2, 0:QC])
        nc.scalar.dma_start(x8_sb[:, 0:2, 1, 0:QC], x8b_r[:, 0:2, 0:QC])
        nc.gpsimd.dma_start(x8_sb[:, 2:KO, 0, 0:QC], x8a_r[:, 2:KO, 0:QC])
        nc.scalar.dma_start(x8_sb[:, 2:KO, 1, 0:QC], x8b_r[:, 2:KO, 0:QC])
        for j in range(1, NJ):
            nc.sync.dma_start(
                wkh_sb[:, :, j * P : (j + 1) * P],
                wkh_r[:, :, j * P : (j + 1) * P])
            nc.sync.dma_start(
                wkl_sb[:, :, j * P : (j + 1) * P],
                wkl_r[:, :, j * P : (j + 1) * P])
        nc.sync.dma_start(tri_sb[:, 0, :], tri)
        nc.gpsimd.memset(bias_sb[:], ACT_BIAS)
        # exp activation-table warm-up: off the critical path
        nc.scalar.activation(warm_sb[:], tri_sb[:, 0, 0:2], EXP,
                             bias=bias_sb[:], scale=ACT_SCALE)
        for j in range(NJ):
            nc.sync.dma_start(
                wqh_sb[:, :, j * P : (j + 1) * P],
                wqh_r[:, :, j * P : (j + 1) * P])
            nc.sync.dma_start(
                wql_sb[:, :, j * P : (j + 1) * P],
                wql_r[:, :, j * P : (j + 1) * P])
            # x8 bulk on Pool queue: deadlines are loose (sc chunk c needed
            # from q-tile 4c) and ACT must stay free for exp from qs 0
            nc.gpsimd.dma_start(
                x8_sb[:, 2 * j : 2 * j + 2, 0, QC:S],
                x8a_r[:, 2 * j : 2 * j + 2, QC:S])
            nc.gpsimd.dma_start(
                x8_sb[:, 2 * j : 2 * j + 2, 1, QC:S],
                x8b_r[:, 2 * j : 2 * j + 2, QC:S])
        for ko in range(KO):
            nc.sync.dma_start(wvh_sb[:, ko, :, :], wvh_r[:, ko, :, :])
        for ko in range(0, KO, 2):
            nc.sync.dma_start(
                wvl_sb[:, ko : ko + 2, :], wvl_r[:, ko : ko + 2, :])
        nc.sync.dma_start(wo_sb[:], wo_r)
        # ones columns (softmax denominator accumulators)
        nc.gpsimd.memset(v8_sb[:, :, 0, :, DK : DK + 1], ONES8)
        nc.gpsimd.memset(v8_sb[:, :, 1, :, DK : DK + 1], 0.0)
        nc.gpsimd.memset(v16_sb[:, :, :, DK : DK + 1], ONES8)
        # zero the dead halves of the first q s-chunk before its drains
        nc.gpsimd.memset(qt_ev[DK:P, :, 0:QC], 0.0)
        nc.gpsimd.memset(qt_od[0:DK, :, 0:QC], 0.0)

        # ---- building blocks ------------------------------------------
        # drains split across ACT and DVE to keep DVE off the critical path
        def drain_k(pj, j, sc):
            nc.vector.tensor_copy(kt_sb[:, j, sc * QC : (sc + 1) * QC], pj[:])

        def drain_q(pj, j, sc):
            sl = slice(sc * QC, (sc + 1) * QC)
            nc.scalar.copy(qt_ev[0:DK, j, sl], pj[0:DK, :])
            nc.vector.tensor_copy(qt_od[DK:P, j, sl], pj[DK:P, :])

        # DoubleRow hi/lo K/Q projection group: out pj [P, QC] accumulates
        # two independent 256-wide half column groups, 12 DR matmuls each.
        def kq_mm(pj, wh_sb, wl_sb, j, sc, half, i):
            c0 = sc * QC + half * HC
            o0 = half * HC
            if i < KO:   # hi matmul, slab i: slots (x_hi, W_hi)+(x_lo, W_hi)
                nc.tensor.matmul(
                    pj[:, o0 : o0 + HC],
                    wh_sb[:, i, j * P : (j + 1) * P]
                        .rearrange("p (one m) -> p one m", one=1)
                        .broadcast_to([P, 2, P]),
                    x8_sb[:, i, :, c0 : c0 + HC],
                    start=(i == 0), stop=False, perf_mode=DR,
                )
            else:        # correction matmul, slab pair: (x_hi, W_lo) x2
                kp = i - KO
                nc.tensor.matmul(
                    pj[:, o0 : o0 + HC],
                    wl_sb[:, 2 * kp : 2 * kp + 2, j * P : (j + 1) * P],
                    x8_sb[:, 2 * kp : 2 * kp + 2, 0, c0 : c0 + HC],
                    start=False, stop=(kp == KO // 2 - 1), perf_mode=DR,
                )

        def emit_kq_group(drain, wh_sb, wl_sb, j, sc):
            pj = mixps.tile([P, QC], F32, tag="mix", name="pj")
            for half in range(2):
                for i in range(KO + KO // 2):
                    kq_mm(pj, wh_sb, wl_sb, j, sc, half, i)
            drain(pj, j, sc)

        # V projection group: same 12-DR-matmul structure per 256-half.
        def v_mm(vp, kc, half, i):
            o0 = half * HC
            if i < KO:
                nc.tensor.matmul(
                    vp[:, o0 : o0 + HC],
                    x8_sb[:, i, :, kc * P : (kc + 1) * P],
                    wvh_sb[:, i, :, o0 : o0 + HC],
                    start=(i == 0), stop=False, perf_mode=DR,
                )
            else:
                kp = i - KO
                nc.tensor.matmul(
                    vp[:, o0 : o0 + HC],
                    x8_sb[:, 2 * kp : 2 * kp + 2, 0, kc * P : (kc + 1) * P],
                    wvl_sb[:, 2 * kp : 2 * kp + 2, o0 : o0 + HC],
                    start=False, stop=(kp == KO // 2 - 1), perf_mode=DR,
                )

        def drain_v(vp, kc):
            # all heads -> f16 at scale 8; heads 0..5 also e4m3 hi/lo at 8
            vph = vp[:, 0 : NHA * DK].rearrange("p (h e) -> p h e", h=NHA)
            nc.vector.tensor_scalar_mul(
                v8_sb[:, kc, 0, :, 0:DK], vph, VS8)
            nc.vector.scalar_tensor_tensor(
                v8_sb[:, kc, 1, :, 0:DK], vph, VS8,
                v8_sb[:, kc, 0, :, 0:DK],
                mybir.AluOpType.mult, mybir.AluOpType.subtract)
            # f16 copy on ACT (activation Copy with scale): keeps DVE cooler
            nc.scalar.activation(
                v16_sb[:, kc, :, 0:DK],
                vp.rearrange("p (h e) -> p h e", h=NH),
                mybir.ActivationFunctionType.Copy, scale=VS8)

        def emit_v_group(kc):
            vp = mixps.tile([P, HD], F32, tag="mix", name="vp")
            for half in range(2):
                for i in range(KO + KO // 2):
                    v_mm(vp, kc, half, i)
            drain_v(vp, kc)

        def drain_o(op, qs, no):
            o2 = osb.tile([P, QC], BF16, tag="o", name="o2")
            if no == 0:
                nc.vector.tensor_copy(o2[:], op[:])
            else:
                nc.scalar.copy(o2[:], op[:])
            nc.sync.dma_start(
                out[qs * P : (qs + 1) * P, no * QC : (no + 1) * QC], o2[:]
            )

        def emit_oproj_half(qs, no):
            op = mixps.tile([P, QC], F32, tag="mix", name="op")
            for j in range(NJ):
                nc.tensor.matmul(
                    op[:],
                    ctxT_sb[:, j, qs, :],
                    wo_sb[:, j, no * QC : (no + 1) * QC],
                    start=(j == 0),
                    stop=(j == NJ - 1),
                )
            drain_o(op, qs, no)

        def emit_oproj(qs):
            for no in range(2):
                emit_oproj_half(qs, no)

        # ---- prologue: first-chunk projections -------------------------
        for j in range(NJ):
            emit_kq_group(drain_k, wkh_sb, wkl_sb, j, 0)
        for j in range(NJ):
            emit_kq_group(drain_q, wqh_sb, wql_sb, j, 0)
        emit_v_group(0)

        # PE filler: later chunks' projections plus deferred output
        # projections, in ~0.3us micro-steps (6 DR matmuls each).
        from collections import deque

        def kq_steps(drain_fn, wh_sb, wl_sb, j, sc, dl):
            st = {}

            def mk(step):
                def f():
                    if step == 0:
                        st["pj"] = mixps.tile([P, QC], F32, tag="mix", name="pj")
                    half = step // 2
                    for i in range(6 * (step % 2), 6 * (step % 2) + 6):
                        kq_mm(st["pj"], wh_sb, wl_sb, j, sc, half, i)
                return f

            def drain():
                drain_fn(st["pj"], j, sc)

            return [(dl, mk(k)) for k in range(4)] + [(dl, drain)]

        def v_steps(kc, dl):
            st = {}

            def mk(step):
                def f():
                    if step == 0:
                        st["vp"] = mixps.tile([P, HD], F32, tag="mix", name="vp")
                    half = step // 2
                    for i in range(6 * (step % 2), 6 * (step % 2) + 6):
                        v_mm(st["vp"], kc, half, i)
                return f

            def drain():
                drain_v(st["vp"], kc)

            return [(dl, mk(k)) for k in range(4)] + [(dl, drain)]

        def op_steps(oqs, no, dl):
            st = {}

            def mk(j0):
                def f():
                    if j0 == 0:
                        st["op"] = mixps.tile([P, QC], F32, tag="mix", name="op")
                    for j in (j0, j0 + 1):
                        nc.tensor.matmul(
                            st["op"][:],
                            ctxT_sb[:, j, oqs, :],
                            wo_sb[:, j, no * QC : (no + 1) * QC],
                            start=(j == 0),
                            stop=(j == NJ - 1),
                        )
                return f

            def drain():
                drain_o(st["op"], oqs, no)

            return [(dl, mk(0)), (dl, mk(2)), (dl, drain)]

        def qzero_step(sc, dl):
            def f():
                nc.gpsimd.memset(qt_ev[DK:P, :, sc * QC : (sc + 1) * QC], 0.0)
                nc.gpsimd.memset(qt_od[0:DK, :, sc * QC : (sc + 1) * QC], 0.0)
            return [(dl, f)]

        steps = deque()
        for sc in (1, 2, 3):
            lo = 4 * sc
            steps.extend(qzero_step(sc, lo - 2))
            for kc in range(lo - 3, lo):
                steps.extend(v_steps(kc, kc))
            for j in range(NJ):
                steps.extend(kq_steps(drain_k, wkh_sb, wkl_sb, j, sc, lo))
            for j in range(NJ):
                steps.extend(kq_steps(drain_q, wqh_sb, wql_sb, j, sc, lo))
            steps.extend(v_steps(lo, lo))
        for kc in (13, 14, 15):
            steps.extend(v_steps(kc, kc))
        for oqs in (0, 1, 2, 3, 4, 6, 7, 8):  # OP(5)/OP(9) held as tail fill
            steps.extend(op_steps(oqs, 0, 99))
            steps.extend(op_steps(oqs, 1, 99))

        triA_b = tri_sb.broadcast_to([P, NHA, P])
        triB_b = tri_sb.broadcast_to([P, NHS, P])

        def emit_norm(qs, cx4):
            # normalize (q is the partition dim -> broadcast along free)
            rr = rrp.tile([P, 2, 4, 1], F32, tag="rr", name="rr")
            nc.vector.reciprocal(rr[:], cx4[:, :, :, DK : DK + 1])
            cn = cnp.tile([P, 2, 4, DK], F16, tag="cn", name="cn")
            nc.vector.tensor_mul(
                cn[:], cx4[:, :, :, 0:DK], rr.broadcast_to([P, 2, 4, DK]))
            return cn

        def emit_transp(qs, cn, eng=None):
            # ctx^T via DMA-engine transposes on the SP queue (no PE cost)
            eng = eng or nc.sync
            for j in range(NJ):
                eng.dma_start_transpose(
                    ctxT_sb[:, j, qs, :],
                    cn[:, (2 * j) // 4, (2 * j) % 4 : (2 * j) % 4 + 2, :])

        def emit_scores(sp, qs, kc, heads):
            for h in heads:
                j = h // 2
                qsrc = qt_ev if h % 2 == 0 else qt_od
                nc.tensor.matmul(
                    sp[:, h, :],
                    kt_sb[:, j, kc * P : (kc + 1) * P],
                    qsrc[:, j, qs * P : (qs + 1) * P],
                    start=True,
                    stop=True,
                    skip_group_check=True,
                )

        # ---- main q-tile sweep ------------------------------------------
        import os
        nqs_lim = int(os.environ.get("NQS_LIM", NQS))
        pending = None  # (qs, cx4) awaiting norm/transpose/output-projection
        kc_done = 0
        for qs in range(nqs_lim):
            nkc = qs + 1
            pcn = None
            if pending is not None:
                pcn = emit_norm(pending[0], pending[1])  # DVE only
            # 4 heads per 512-f32 PSUM bank (65-wide groups must not cross a
            # bank boundary): head h lives at cx4[:, h//4, h%4, :].  The
            # banks are zeroed by the start=True writes of the first PV
            # matmul into each bank (h=0 and h=4 at kc=0).
            cx = cxps.tile([P, 2, 512], F32, tag="cx", name="cx")
            cx4 = cx[:, :, 0 : 4 * (DK + 1)].rearrange(
                "p b (h e) -> p b h e", h=4)
            # force any filler whose deadline has arrived
            while steps and steps[0][0] <= qs:
                steps.popleft()[1]()
            for kc in range(nkc):
                sp = spps.tile([P, NH, P], F32, tag="sp", name="sp")
                diag = kc == qs
                emit_scores(sp, qs, kc, range(NH))
                # heads 6-7: Schraudolph on DVE, every tile (uniform scale)
                ptB = ptp.tile([P, NHS, P], F16, tag="ptB", name="ptB")
                nc.vector.tensor_scalar(
                    ptB[:].bitcast(I16), sp[:, NHA:NH, :],
                    SCH_A, SCH_B,
                    mybir.AluOpType.mult, mybir.AluOpType.add)
                if diag:
                    # diagonal tile: heads 0-5 exact f16 exp (guarantees a
                    # nonzero denominator for every row — e4m3 pt can flush
                    # an entire early row to zero) + all-f16 P@V
                    ptD = ptp.tile([P, NHA, P], F16, tag="ptD", name="ptD")
                    nc.scalar.activation(ptD[:], sp[:, 0:NHA, :], EXP,
                                         bias=bias_sb[:], scale=ACT_SCALE)
                    # all-SBUF multiplies: run on Pool
                    nc.gpsimd.tensor_mul(ptD[:], ptD[:], triA_b)
                    nc.gpsimd.tensor_mul(ptB[:], ptB[:], triB_b)
                else:
                    ptA = ptp.tile([P, NHA, P], E4, tag="ptA", name="ptA")
                    nc.scalar.activation(ptA[:], sp[:, 0:NHA, :], EXP,
                                         bias=bias_sb[:], scale=ACT_SCALE)
                inline_op = False
                if pending is not None:
                    # deferred transpose/oproj: transposes issue right at
                    # the tile start (their ~1.5us DMA-queue latency rides
                    # under the first kc steps), oprojs 3-4 kc steps later
                    if kc == 0:
                        emit_transp(pending[0], pcn)
                    if pending[0] >= 10:
                        if kc == 3:
                            emit_oproj_half(pending[0], 0)
                            inline_op = True
                        if kc == 4:
                            emit_oproj_half(pending[0], 1)
                            inline_op = True
                # filler micro-steps, paced so the queue lasts the whole
                # sweep (a dry queue leaves PE idling at the exp rate in
                # the late tiles); deadlines force correctness
                kc_done += 1
                want = (len(steps) * 8) // max(8 * (136 - kc_done), 1)
                want = min(3, max(1, want)) + (1 if kc == 0 else 0)
                if inline_op:
                    want = 0
                for _ in range(want):
                    if steps:
                        steps.popleft()[1]()
                for h in range(NH):
                    if diag and h < NHA:
                        nc.tensor.matmul(
                            cx4[:, h // 4, h % 4, :],
                            ptD[:, h, :],
                            v16_sb[:, kc, h, :],
                            start=(kc == 0 and h % 4 == 0),
                            stop=(kc == nkc - 1),
                            skip_group_check=True,
                        )
                    elif h < NHA:
                        nc.tensor.matmul(
                            cx4[:, h // 4, h % 4, :],
                            ptA[:, h, :]
                                .rearrange("p (one m) -> p one m", one=1)
                                .broadcast_to([P, 2, P]),
                            v8_sb[:, kc, :, h, :],
                            start=(kc == 0 and h % 4 == 0),
                            stop=(kc == nkc - 1),
                            perf_mode=DR,
                            skip_group_check=True,
                        )
                    else:
                        nc.tensor.matmul(
                            cx4[:, h // 4, h % 4, :],
                            ptB[:, h - NHA, :],
                            v16_sb[:, kc, h, :],
                            start=(kc == 0 and h % 4 == 0),
                            stop=(kc == nkc - 1),
                            skip_group_check=True,
                        )
            pending = (qs, cx4)
        # tail: last q-tile's norm/transpose/projection, with the held-back
        # OP(5) (plus any queue remainder) giving PE work while the DVE norm
        # chain and the SP transposes land
        if pending is not None and nqs_lim == NQS:
            cn15 = emit_norm(pending[0], pending[1])
            while steps:
                steps.popleft()[1]()
            emit_oproj_half(5, 0)
            emit_transp(pending[0], cn15)
            emit_oproj_half(5, 1)
            emit_oproj(9)
            emit_oproj(pending[0])


def build_nc():
    nc = bacc.Bacc("TRN2", target_bir_lowering=False, debug=False)
    x8a = nc.dram_tensor("x8a", [D, S], E4, kind="ExternalInput").ap()
    x8b = nc.dram_tensor("x8b", [D, S], E4, kind="ExternalInput").ap()
    wqh = nc.dram_tensor("wqh", [D, HD], E4, kind="ExternalInput").ap()
    wql = nc.dram_tensor("wql", [D, HD], E4, kind="ExternalInput").ap()
    wkh = nc.dram_tensor("wkh", [D, HD], E4, kind="ExternalInput").ap()
    wkl = nc.dram_tensor("wkl", [D, HD], E4, kind="ExternalInput").ap()
    wvh = nc.dram_tensor("wvh", [D, 2, HD], E4, kind="ExternalInput").ap()
    wvl = nc.dram_tensor("wvl", [D, HD], E4, kind="ExternalInput").ap()
    wo = nc.dram_tensor("wo", [HD, D], F16, kind="ExternalInput").ap()
    tri = nc.dram_tensor("tri", [P, P], F16, kind="ExternalInput").ap()
    out = nc.dram_tensor("out", [S, D], BF16, kind="ExternalOutput").ap()
    with tile.TileContext(nc) as tc:
        with ExitStack() as ctx:
            with nc.allow_low_precision(reason="fp16/fp8 kernel by design"):
                _emit(ctx, tc, x8a, x8b, wqh, wql, wkh, wkl, wvh, wvl, wo,
                      tri, out)
    nc.compile()
    return nc


def _split_e4(t, scale):
    """hi/lo e4m3 split of t*scale (host-side, round-to-nearest)."""
    import ml_dtypes
    E4n = ml_dtypes.float8_e4m3
    tf = np.asarray(t, np.float32) * scale
    hi = tf.astype(E4n)
    lo = (tf - hi.astype(np.float32)).astype(E4n)
    return hi, lo


def make_in_maps(x, W_q, W_k, W_v, W_o):
    import ml_dtypes
    E4n = ml_dtypes.float8_e4m3

    x = np.asarray(x, dtype=np.float32)
    WqT = np.ascontiguousarray(np.asarray(W_q, np.float32).T)
    WkT = np.ascontiguousarray(np.asarray(W_k, np.float32).T)
    WvT = np.ascontiguousarray(np.asarray(W_v, np.float32).T)
    WoT = np.ascontiguousarray(np.asarray(W_o, np.float32).T).astype(
        np.float16)
    # tri[k, q] = 1 where q >= k (within a diagonal 128x128 block)
    tri = np.triu(np.ones((P, P), np.float32)).astype(np.float16)
    in_maps = []
    for c in range(2 * B):
        b, g = c // 2, c % 2
        xh, xl = _split_e4(x[b].T, XS)               # [D, S]
        wqh, wql = _split_e4(WqT[:, g * HD : (g + 1) * HD], WS)
        wkh, wkl = _split_e4(WkT[:, g * HD : (g + 1) * HD], WS)
        wvh, wvl = _split_e4(WvT[:, g * HD : (g + 1) * HD], WS)
        wvh2 = np.ascontiguousarray(
            np.stack([wvh, wvh], axis=1))            # [D, 2, HD]
        in_maps.append({
            "x8a": np.ascontiguousarray(xh),
            "x8b": np.ascontiguousarray(xl),
            "wqh": np.ascontiguousarray(wqh),
            "wql": np.ascontiguousarray(wql),
            "wkh": np.ascontiguousarray(wkh),
            "wkl": np.ascontiguousarray(wkl),
            "wvh": wvh2,
            "wvl": np.ascontiguousarray(wvl),
            "wo": np.ascontiguousarray(WoT[g * HD : (g + 1) * HD, :]),
            "tri": tri,
        })
    return in_maps


def get_runner():
    """Build (once) and cache a jitted 8-core executor for the bass program.

    Returns run(in_maps) -> list of per-core {name: np.ndarray} outputs.
    Mirrors concourse.bass2jax.run_bass_via_pjrt but caches the jitted
    callable so repeat kernel() calls skip re-lowering/compiling.
    """
    if "runner" in _CACHE:
        return _CACHE["runner"]
    import jax
    from jax.experimental.shard_map import shard_map
    from jax.sharding import Mesh, PartitionSpec
    from concourse import mybir as _mb
    from concourse.bass2jax import (
        _bass_exec_p, install_neuronx_cc_hook, partition_id_tensor)

    install_neuronx_cc_hook()
    nc = build_nc()
    n_cores = 2 * B

    partition_name = (nc.partition_id_tensor.name
                      if nc.partition_id_tensor else None)
    in_names, out_names, out_avals = [], [], []
    for alloc in nc.m.functions[0].allocations:
        if not isinstance(alloc, _mb.MemoryLocationSet):
            continue
        name = alloc.memorylocations[0].name
        if alloc.kind == "ExternalInput":
            if name != partition_name:
                in_names.append(name)
        elif alloc.kind == "ExternalOutput":
            out_names.append(name)
            out_avals.append(jax.core.ShapedArray(
                tuple(alloc.tensor_shape), _mb.dt.np(alloc.dtype)))
    n_params = len(in_names)
    all_names = in_names + out_names
    if partition_name is not None:
        all_names = all_names + [partition_name]

    def _body(*args):
        operands = list(args)
        if partition_name is not None:
            operands.append(partition_id_tensor())
        outs = _bass_exec_p.bind(
            *operands,
            out_avals=tuple(out_avals),
            in_names=tuple(all_names),
            out_names=tuple(out_names),
            lowering_input_output_aliases=(),
            sim_require_finite=False,
            sim_require_nnan=False,
            nc=nc,
        )
        return tuple(outs)

    devices = jax.devices()[:n_cores]
    mesh = Mesh(np.asarray(devices), ("core",))
    n_outs = len(out_names)
    sharded = jax.jit(
        shard_map(
            _body, mesh=mesh,
            in_specs=(PartitionSpec("core"),) * (n_params + n_outs),
            out_specs=(PartitionSpec("core"),) * n_outs,
            check_rep=False,
        ),
        donate_argnums=tuple(range(n_params, n_params + n_outs)),
        keep_unused=True,
    )

    def run(in_maps, device_arrays=None):
        concat_in = device_arrays if device_arrays is not None else [
            np.concatenate([np.asarray(in_maps[c][i_name])
                            for c in range(n_cores)], axis=0)
            for i_name in in_names
        ]
        concat_zeros = [
            np.zeros((n_cores * av.shape[0], *av.shape[1:]), av.dtype)
            for av in out_avals
        ]
        out_arrs = sharded(*concat_in, *concat_zeros)
        return [
            {name: np.asarray(out_arrs[i]).reshape(
                n_cores, *out_avals[i].shape)[c]
             for i, name in enumerate(out_names)}
            for c in range(n_cores)
        ]

    _CACHE["runner"] = (run, in_names, out_avals)
    return _CACHE["runner"]


def _run_cores(in_maps):
    """Execute the 8-core program; returns per-core {name: np.ndarray}."""
    from concourse._compat import axon_active
    if axon_active():
        # remote-accelerator proxy: use the cached jitted PJRT executor so
        # repeat calls skip re-lowering/compiling
        run, _, _ = get_runner()
        return run(in_maps)
    # native path (local /dev/neuron*): run_bass_kernel_spmd handles NEFF
    # compile caching + device execution directly
    if "nc" not in _CACHE:
        _CACHE["nc"] = build_nc()
    res = run_bass_kernel_spmd(_CACHE["nc"], in_maps, core_ids=list(range(2 * B)))
    _CACHE["last_exec_time_ns"] = res.exec_time_ns
    return res.results


def kernel(x, W_q, W_k, W_v, W_o):
    in_maps = make_in_maps(x, W_q, W_k, W_v, W_o)
    results = _run_cores(in_maps)
    out = np.empty((B, S, D), np.float32)
    for b in range(B):
        out[b] = (results[2 * b]["out"].astype(np.float32)
                  + results[2 * b + 1]["out"].astype(np.float32))
    return out
